# revision 37
# baseline (speedup 1.0000x reference)
"""Trainium2 Bass kernel for AngularSymmetryMod (ANI-style angular symmetry functions).

Math: out[b,i,l] = sum_{j,k} (1+lam*cos(theta-theta_t))^zeta * exp(-ita*((R_ij+R_ik)/2-Rs)^2)
                            * f_ij*f_ik * 2^(1-zeta)
over a 40-point parameter grid l=(lam in {+-1}, 5 Rs values, 4 theta_t values), zeta=4.

Key optimizations over the gathered-pair baseline (47us -> ~27.4us):
 1. Cyclic pair enumeration: pair (j, k=(j+m) mod 32) for m=0..16, j split 16/16 across
    partition halves. All per-pair operand reads become AFFINE access patterns over compact
    per-partition rows: k-side operands are sliding windows Dk[j'+m] of a rotated 32-col row,
    j-side operands are 0-stride broadcasts of a 16-col row. Input shrinks ~12x (no host
    inflation of the pair layout), killing the 4.3us input-DMA wall the baseline had.
    Weights: m=0 (diag) 1x, m=1..15 2x (unordered pair symmetry), m=16 1x (enumerated twice);
    the global 2/8 factor rides in rad0's exp bias, the m in {0,16} halving in two tiny ops.
 2. theta_t = quadrants -> the angular factor collapses to 4 fields (1+-c)^4, (1+-s)^4;
    the range reduction is split so SIN(sin-half) fires before the cos-half is reduced.
 3. Radial Gaussians via an unscaled recurrence W_{r+1} = W_r * exp(ita*dR_r*q) (pure bf16
    2x-rate multiplies; only 3 Exp activations total); the per-step constants are folded
    into the reduce-stage scalars / ACT accumulate scales (GAM).
 4. One activation-table load per function family (Exp block fully before the Sin block).
 5. Engine economics learned from traces: Pool(GpSimd) big ops inflate concurrent DVE ops
    ~2x (SBUF contention) -> Pool does nothing; tensor_scalar runs 2x dual-pipe for f32,
    tensor_tensor runs 2x only for 16-bit dtypes, scalar_tensor_tensor never does.
    The dot-product suite runs in f16 (11-bit mantissa keeps theta error acceptable at
    7.7e-3 total, 2x rate); cutoffs/fields/W/reduce products in bf16; d, reciprocal and
    range reduction in f32 to track the f32 reference through near-singular denominators.
 6. The 20 fused multiply-reduces: 12 as DVE scalar_tensor_tensor with accum_out, 8 (r<2)
    as DVE bf16 products accumulated by the otherwise-idle ACT engine (Copy + accum_out
    into a PSUM accumulator) - measured balance point of the two engines' tails.
 7. Final 40-column assembly + cross-half pair sum via 3 bf16 PE matmuls of a 0/1 pairing
    matrix (bf16 avoids the f32 double-pass LDWEIGHTS).

Sharding: data-parallel over batch (16 molecules -> 2 per core on 8 cores). No collectives.
Layout per core: 128 partitions = (jhalf:2, b_loc:2, i:32), free = (m:17, j':16) = 272.
"""

import sys
import numpy as np

sys.path.insert(0, "/opt/trn_rl_repo")

from contextlib import ExitStack

import concourse.bass as bass
import concourse.tile as tile
from concourse import bacc, mybir
from concourse.ap import AP
from concourse.bass_utils import run_bass_kernel_spmd

B, N, L = 16, 32, 40
NCORES = 8
B_LOC = B // NCORES  # 2
P = 128              # partitions = 2 halves * B_LOC * N
MC = 17              # m blocks (cyclic shift distances 0..16)
JH = 16              # j' per partition-half
NT = MC * JH         # 272 free elements per partition

BOHR = 0.52917721092
ITA = 1.12
RS = (np.array([0.5, 1.17, 1.83, 2.5, 3.17]) / BOHR).astype(np.float64)
TWO_PI = float(2.0 * np.pi)
RC = float(12582912.0)  # 1.5 * 2^23 f32 round-to-int magic constant

# radial chain constants: rad_{r+1} = rad_r * exp(ITA*dR_r*q) * CCH[r]
DR = RS[1:] - RS[:-1]                      # [0]==[2]==[3], [1] differs
KA = float(ITA * DR[0])
KB = float(ITA * DR[1])
CCH = [float(np.exp(-ITA * DR[r] * (RS[r] + RS[r + 1]))) for r in range(4)]
GAM = [1.0]
for r in range(4):
    GAM.append(GAM[-1] * CCH[r])  # fold chain constants into the reduce stage
ECH = [0, 1, 0, 0]  # which step field (Ea/Eb) each chain step uses

# f32 input column offsets
OFF_DJ = 0
OFF_DK = 16
OFF_CI = 48
OFF_CJK = 51  # 3 x [Cj_c(16) | Ck_c(32)] interleaved per coordinate
NIN = 195

F32 = mybir.dt.float32
BF16 = mybir.dt.bfloat16
F16 = mybir.dt.float16
OP = mybir.AluOpType
ACT = mybir.ActivationFunctionType


def _win(t, col_off, m_stride, m_cnt=MC, j_cnt=JH):
    """Affine (m, j') access pattern over a compact per-partition row of tile t.
    m_stride=1 -> sliding window (k-side); m_stride=0 -> broadcast (j-side)."""
    base = t[:]
    part = list(base.ap[0])
    return AP(base.tensor, base.offset + col_off, [part, [m_stride, m_cnt], [1, j_cnt]])


def _build():
    nc = bacc.Bacc("TRN2", target_bir_lowering=False, debug=False)
    inp_d = nc.declare_dram_parameter("inp", [P, NIN], F32, isOutput=False)
    inpb_d = nc.declare_dram_parameter("inpb", [P, 48], BF16, isOutput=False)
    cst_d = nc.declare_dram_parameter("cst", [P, 64], BF16, isOutput=False)
    out_d = nc.declare_dram_parameter("out", [B_LOC * N, L], F32, isOutput=True)

    with tile.TileContext(nc) as tc, ExitStack() as ctx:
        pool = ctx.enter_context(tc.tile_pool(name="sb", bufs=1))
        scr_pool = ctx.enter_context(tc.tile_pool(name="scr", bufs=6))
        prod_pool = ctx.enter_context(tc.tile_pool(name="prod", bufs=8))
        aout_pool = ctx.enter_context(tc.tile_pool(name="aout", bufs=4))
        psum = ctx.enter_context(tc.tile_pool(name="ps", bufs=1, space="PSUM"))

        def big(tag, dt=F32):
            return pool.tile([P, MC, JH], dt, name=tag, tag=tag)

        raw = pool.tile([P, NIN], F32, name="raw", tag="raw")
        rawb = pool.tile([P, 48], BF16, name="rawb", tag="rawb")
        cst = pool.tile([P, 64], BF16, name="cst", tag="cst")
        # all DMA kicks on the sync sequencer; Pool runs nothing at all
        # (concurrent Pool big-ops inflate DVE op latency ~2x on this HW)
        nc.sync.dma_start(raw[:, 0:OFF_CI], inp_d[:, 0:OFF_CI])
        nc.sync.dma_start(raw[:, OFF_CI:NIN], inp_d[:, OFF_CI:NIN])
        nc.sync.dma_start(rawb[:], inpb_d[:])
        nc.sync.dma_start(cst[:], cst_d[:])

        ci = [raw[:, OFF_CI + c : OFF_CI + c + 1] for c in range(3)]
        Dj_b = _win(raw, OFF_DJ, 0)
        Dk_w = _win(raw, OFF_DK, 1)
        Fj_b = _win(rawb, 0, 0)
        Fk_w = _win(rawb, 16, 1)

        # bias tiles
        b_mrs0 = pool.tile([P, 1], F32, name="b_mrs0", tag="b_mrs0")
        nc.vector.memset(b_mrs0[:], float(-RS[0]))
        b_l4 = pool.tile([P, 1], F32, name="b_l4", tag="b_l4")
        nc.vector.memset(b_l4[:], float(np.log(0.25)))
        b_one = pool.tile([P, 1], F32, name="b_one", tag="b_one")
        nc.vector.memset(b_one[:], 1.0)
        halfc = pool.tile([P, 16], BF16, name="halfc", tag="halfc")
        nc.vector.memset(halfc[:], 0.5)

        # ---------------- DVE: q first (feeds ACT), then theta path ----------------
        q3 = big("q3")
        nc.vector.tensor_tensor(q3[:], Dj_b, Dk_w, OP.add)
        Vjk = pool.tile([P, 3, 48], F16, name="Vjk", tag="Vjk")
        for c in range(3):
            nc.vector.tensor_scalar(
                Vjk[:, c, :], raw[:, OFF_CJK + 48 * c : OFF_CJK + 48 * (c + 1)],
                ci[c], None, OP.subtract)
        base = Vjk[:]
        part = list(base.ap[0])
        vj3 = AP(base.tensor, base.offset, [part, [48, 3], [0, MC], [1, JH]])
        vk3 = AP(base.tensor, base.offset + 16, [part, [48, 3], [1, MC], [1, JH]])
        da3 = pool.tile([P, 3, MC, JH], F16, name="da3", tag="da3")
        nc.vector.tensor_tensor(da3[:], vj3, vk3, OP.mult)
        den = big("den")
        nc.vector.tensor_tensor(den[:], Dj_b, Dk_w, OP.mult)
        dxy = big("dxy", F16)
        nc.vector.tensor_tensor(dxy[:], da3[:, 0], da3[:, 1], OP.add)
        dot = big("dot", F16)
        nc.vector.tensor_tensor(dot[:], dxy[:], da3[:, 2], OP.add)

        denp = big("denp")
        nc.vector.tensor_scalar(denp[:], den[:], 1e-5, TWO_PI, OP.add, OP.mult)
        rden = big("rden")
        nc.vector.reciprocal_approx_fast(rden[:], denp[:])
        ths = big("ths")
        nc.vector.tensor_tensor(ths[:], dot[:], rden[:], OP.mult)
        nfs = big("nfs")
        nc.vector.tensor_scalar(nfs[:], ths[:], RC, RC, OP.add, OP.subtract)
        frs = big("frs")
        nc.vector.tensor_tensor(frs[:], ths[:], nfs[:], OP.subtract)
        thc = big("thc")
        nc.vector.tensor_scalar(thc[:], ths[:], 0.25, None, OP.add)
        nfc = big("nfc")
        nc.vector.tensor_scalar(nfc[:], thc[:], RC, RC, OP.add, OP.subtract)
        frc = big("frc")
        nc.vector.tensor_tensor(frc[:], thc[:], nfc[:], OP.subtract)

        # ---------------- ACT: exp-family block, then trig-family ----------------
        sq0 = big("sq0")
        nc.scalar.activation(sq0[:], q3[:], ACT.Square, bias=b_mrs0[:], scale=0.5)
        rad0 = big("rad0", BF16)
        nc.scalar.activation(rad0[:], sq0[:], ACT.Exp, scale=float(-ITA), bias=b_l4[:])
        Ea = big("Ea", BF16)
        nc.scalar.activation(Ea[:], q3[:], ACT.Exp, scale=KA)
        Eb = big("Eb", BF16)
        nc.scalar.activation(Eb[:], q3[:], ACT.Exp, scale=KB)
        # trig family (one table switch); field order f: 0=cp, 1=sp, 2=cm, 3=sm
        scs = big("scs")
        nc.scalar.activation(scs[:], frs[:], ACT.Sin, scale=TWO_PI)
        g2 = [None] * 4
        for f in (1, 3):
            g2[f] = big(f"g2{f}", BF16)
            nc.scalar.activation(g2[f][:], scs[:], ACT.Square,
                                 bias=b_one[:], scale=(1.0 if f == 1 else -1.0))
        scc = big("scc")
        nc.scalar.activation(scc[:], frc[:], ACT.Sin, scale=TWO_PI)
        for f in (0, 2):
            g2[f] = big(f"g2{f}", BF16)
            nc.scalar.activation(g2[f][:], scc[:], ACT.Square,
                                 bias=b_one[:], scale=(1.0 if f == 0 else -1.0))

        # ---------------- DVE: cut + unscaled W chain (fills the SIN window) ----------------
        cut = big("cut", BF16)
        nc.vector.tensor_tensor(cut[:], Fj_b, Fk_w, OP.mult)
        nc.vector.tensor_tensor(cut[:, 0, :], cut[:, 0, :], halfc[:], OP.mult)
        nc.vector.tensor_tensor(cut[:, 16, :], cut[:, 16, :], halfc[:], OP.mult)
        Ws = [big(f"w{r}", BF16) for r in range(5)]
        nc.vector.tensor_tensor(Ws[0][:], cut[:], rad0[:], OP.mult)
        for r in range(4):
            E = Eb if ECH[r] else Ea
            nc.vector.tensor_tensor(Ws[r + 1][:], Ws[r][:], E[:], OP.mult)

        # ---------------- fields + 20 multiply-reduces (DVE + 8 ACT accums) ----------------
        spart = pool.tile([P, 24], F32, name="spart", tag="spart")
        spartP = psum.tile([P, 24], F32, name="spartP", tag="spartP")
        nc.vector.memset(spart[:], 0.0)
        nc.vector.memset(spartP[:], 0.0)
        av = [big(f"a{f}", BF16) for f in range(4)]
        act_prods = []

        def dve_square(f):
            nc.vector.tensor_tensor(av[f][:], g2[f][:], g2[f][:], OP.mult)

        def dve_red(r, f):
            scr = scr_pool.tile([P, MC, JH], BF16, name=f"scr{r}{f}", tag="scr")
            nc.vector.scalar_tensor_tensor(
                scr[:], Ws[r][:], GAM[r], av[f][:], OP.mult, OP.mult,
                accum_out=spart[:, r * 4 + f : r * 4 + f + 1])

        def dve_prod(r, f):
            pr = prod_pool.tile([P, MC, JH], BF16, name=f"dp{r}{f}", tag="prod")
            nc.vector.tensor_tensor(pr[:], Ws[r][:], av[f][:], OP.mult)
            act_prods.append((r, f, pr))

        for f in (3, 1, 0, 2):  # field readiness order: sm, sp (sin), cp, cm (cos)
            dve_square(f)
            dve_prod(0, f)
            dve_prod(1, f)
            for r in range(2, 5):
                dve_red(r, f)
        # ACT: accumulate the 4 outsourced r=0 products (GAM[0]=1)
        for r, f, pr in act_prods:
            scr2 = aout_pool.tile([P, MC, JH], BF16, name=f"ac{r}{f}", tag="aout")
            nc.scalar.activation(scr2[:], pr[:], ACT.Copy, scale=GAM[r],
                                 accum_out=spartP[:, r * 4 + f : r * 4 + f + 1])

        # ---------------- combine: half pair-sum + 40-column assembly ----------------
        spb = pool.tile([P, 24], BF16, name="spb", tag="spb")
        nc.vector.scalar_tensor_tensor(spb[:], spart[:], 0.0, spartP[:],
                                       OP.add, OP.add)
        s2p = psum.tile([64, L], F32, name="s2p", tag="s2p")
        sp3 = spb[:, 0:20].rearrange("p (r t) -> p r t", r=5, t=4)
        nc.tensor.matmul(s2p[:, 0:20], cst[:, 0:64], spb[:, 0:20])
        o3 = s2p[:].rearrange("n (g r t) -> n g r t", g=2, r=5, t=4)
        nc.tensor.matmul(o3[:, 1, :, 0:2], cst[:, 0:64], sp3[:, :, 2:4])
        nc.tensor.matmul(o3[:, 1, :, 2:4], cst[:, 0:64], sp3[:, :, 0:2])
        s2s = pool.tile([64, L], F32, name="s2s", tag="s2s")
        nc.vector.tensor_copy(s2s[:], s2p[:])
        nc.sync.dma_start(out_d[:], s2s[:])

    nc.compile()
    return nc


def _ensure_ntff_hook():
    """Register the axon NTFF profiling hook if the image lacks antenv.axon_hooks."""
    import types

    try:
        from antenv.axon_hooks import get_axon_ntff_profile_hook
        if get_axon_ntff_profile_hook() is not None:
            return
        have_mod = True
    except ImportError:
        have_mod = False
    try:
        if "/root/.axon_site" not in sys.path:
            sys.path.insert(0, "/root/.axon_site")
        from trn_agent_boot.trn_boot import _ntff_profile_via_ctypes

        hook = _ntff_profile_via_ctypes("/opt/axon/libaxon_pjrt.so")
        if hook is None:
            return
    except Exception:
        return
    if have_mod:
        from antenv import axon_hooks
        axon_hooks.set_axon_ntff_profile_hook(hook)
    else:
        m = types.ModuleType("antenv.axon_hooks")
        _h = [hook]
        m.get_axon_ntff_profile_hook = lambda: _h[0]
        m.set_axon_ntff_profile_hook = lambda h: _h.__setitem__(0, h)
        import antenv
        antenv.axon_hooks = m
        sys.modules["antenv.axon_hooks"] = m


_NC = None


def _get_nc():
    global _NC
    if _NC is None:
        _NC = _build()
    return _NC


# static gather indices (host pack is pure gather/replication of raw inputs)
_pp = np.arange(P)
_H = _pp // 64           # partition half -> j base 16h
_Bp = (_pp // 32) % 2    # local molecule
_Ip = _pp % 32           # atom i
_JBASE = 16 * _H
_JIDX = (_JBASE[:, None] + np.arange(JH)[None, :])            # [P,16] j = 16h+j'
_KIDX = (_JBASE[:, None] + np.arange(32)[None, :]) % 32       # [P,32] rotated k row


def _host_pack(d_cutoff, d, atom_coordinates):
    import ml_dtypes

    d = np.ascontiguousarray(d, dtype=np.float32)
    fc = np.ascontiguousarray(d_cutoff, dtype=np.float32)
    xs = np.ascontiguousarray(atom_coordinates, dtype=np.float32)

    in_maps = []
    for core in range(NCORES):
        dd = d[core * B_LOC : (core + 1) * B_LOC]
        ff = fc[core * B_LOC : (core + 1) * B_LOC]
        xx = xs[core * B_LOC : (core + 1) * B_LOC]
        buf = np.empty((P, NIN), dtype=np.float32)
        buf[:, OFF_CI : OFF_CI + 3] = xx[_Bp, _Ip]
        buf[:, OFF_DJ : OFF_DJ + JH] = dd[_Bp[:, None], _Ip[:, None], _JIDX]
        buf[:, OFF_DK : OFF_DK + 32] = dd[_Bp[:, None], _Ip[:, None], _KIDX]
        for c in range(3):
            buf[:, OFF_CJK + 48 * c : OFF_CJK + 48 * c + JH] = xx[_Bp[:, None], _JIDX, c]
            buf[:, OFF_CJK + 48 * c + JH : OFF_CJK + 48 * (c + 1)] = xx[_Bp[:, None], _KIDX, c]
        bufb = np.empty((P, 48), dtype=np.float32)
        bufb[:, 0:16] = ff[_Bp[:, None], _Ip[:, None], _JIDX]
        bufb[:, 16:48] = ff[_Bp[:, None], _Ip[:, None], _KIDX]
        in_maps.append({
            "inp": buf,
            "inpb": bufb.astype(ml_dtypes.bfloat16),
            "cst": _const_blob(),
        })
    return in_maps


_CST = None


def _const_blob():
    global _CST
    if _CST is None:
        import ml_dtypes
        cst = np.zeros((P, 64), dtype=np.float32)
        cst[np.arange(P), np.arange(P) % 64] = 1.0
        _CST = cst.astype(ml_dtypes.bfloat16)
    return _CST


def kernel(d_cutoff, d, atom_coordinates, _trace=False):
    if _trace:
        _ensure_ntff_hook()
    nc = _get_nc()
    in_maps = _host_pack(d_cutoff, d, atom_coordinates)
    res = run_bass_kernel_spmd(nc, in_maps, core_ids=list(range(NCORES)), trace=_trace)
    out = np.concatenate(
        [res.results[c]["out"].reshape(B_LOC, N, L) for c in range(NCORES)], axis=0
    ).astype(np.float32)
    if _trace:
        kernel._last_results = res
    return out
